# revision 1
# baseline (speedup 1.0000x reference)
"""Bass/Trainium2 kernel for framed 2-layer BiLSTM (nn_BLSTM).

Data-parallel over the 80 framed sequences: 10 per core on 8 NeuronCores.
Each core runs the full network on its shard: input projections (batched
matmuls), both LSTM directions per layer (interleaved recurrences), final
linear. Host does framing/unframing/skip-add only.
"""
import os
import sys
import numpy as np

sys.path.insert(0, "/opt/trn_rl_repo")

import concourse.bass as bass  # noqa: E402
import concourse.mybir as mybir  # noqa: E402
from concourse import bacc  # noqa: E402
from concourse.tile import TileContext  # noqa: E402
from concourse.masks import make_identity  # noqa: E402
from concourse.bass_utils import run_bass_kernel_spmd  # noqa: E402

F32 = mybir.dt.float32
F32R = mybir.dt.float32r

DIM = 768
H = 768
G = 4 * H            # 3072, gate order reordered to [i, f, o, g]
B, T = 4, 2000
WIDTH, STRIDE = 200, 100
NFR = 20             # frames per batch element
NSEQ = B * NFR       # 80
NCORES = 8
SEQ_PC = NSEQ // NCORES   # 10
ROWS = SEQ_PC * WIDTH     # 2000 rows per core
MT = ROWS // 128 + (1 if ROWS % 128 else 0)  # 16 m-tiles (2000 = 15*128 + 80)

_CACHE = {}


def _build_program():
    nc = bacc.Bacc("TRN2", target_bir_lowering=False, debug=False,
                   num_devices=NCORES)

    xfT_d = nc.declare_dram_parameter("xfT", [DIM, ROWS], F32, isOutput=False)
    wx0_d = nc.declare_dram_parameter("wx0", [2, DIM, G], F32, isOutput=False)
    wh0_d = nc.declare_dram_parameter("wh0", [2, H, G], F32, isOutput=False)
    b0_d = nc.declare_dram_parameter("b0", [2, G], F32, isOutput=False)
    wx1_d = nc.declare_dram_parameter("wx1", [2, 2 * H, G], F32, isOutput=False)
    wh1_d = nc.declare_dram_parameter("wh1", [2, H, G], F32, isOutput=False)
    b1_d = nc.declare_dram_parameter("b1", [2, G], F32, isOutput=False)
    linw_d = nc.declare_dram_parameter("linw", [2 * H, DIM], F32, isOutput=False)
    linb_d = nc.declare_dram_parameter("linb", [DIM], F32, isOutput=False)
    out_d = nc.declare_dram_parameter("out", [ROWS, DIM], F32, isOutput=True)

    xw0_d = nc.dram_tensor("xw0", [2, ROWS, G], F32)
    xw1_d = nc.dram_tensor("xw1", [2, ROWS, G], F32)
    ys0_d = nc.dram_tensor("ys0", [ROWS, 2 * H], F32)
    ys1_d = nc.dram_tensor("ys1", [ROWS, 2 * H], F32)
    ysT_d = nc.dram_tensor("ysT", [2 * H, ROWS], F32R)

    def mrows(m):
        return min(128, ROWS - m * 128)

    with TileContext(nc) as tc:
        with tc.tile_pool(name="const", bufs=1) as constp:
            ident = constp.tile([128, 128], F32)
            make_identity(nc, ident[:])
            ones = constp.tile([1, 128], F32)
            nc.vector.memset(ones[:], 1.0)

            # ---------- batched input projection xw = x @ Wx + b ----------
            def proj_phase(kt, lhsT_load, wx_dram, b_dram, xw_dram):
                """kt: number of 128-K tiles; lhsT_load(m, lt): fill lt tile
                with the (128k x 128m) lhsT tiles for m-tile m."""
                for d in range(2):
                    with tc.tile_pool(name="wxp", bufs=1) as wxp, \
                         tc.tile_pool(name="bbp", bufs=1) as bbp, \
                         tc.tile_pool(name="pp", bufs=4, space="PSUM") as pp, \
                         tc.tile_pool(name="lt", bufs=2) as ltp, \
                         tc.tile_pool(name="xo", bufs=2) as xop:
                        wx_sb = wxp.tile([128, kt, G], F32R)
                        for k in range(kt):
                            nc.sync.dma_start(
                                wx_sb[:, k],
                                wx_dram[d, k * 128:(k + 1) * 128, :].bitcast(F32R))
                        bsb = bbp.tile([1, G], F32)
                        nc.sync.dma_start(bsb[:], b_dram[d][None, :])
                        bb = bbp.tile([128, G], F32)
                        for n in range(6):
                            ns = slice(n * 512, (n + 1) * 512)
                            bps = pp.tile([128, 512], F32, tag="pp")
                            nc.tensor.matmul(bps[:], ones[:], bsb[:, ns],
                                             start=True, stop=True)
                            nc.vector.tensor_copy(bb[:, ns], bps[:])
                        for m in range(MT):
                            mr = mrows(m)
                            lt = ltp.tile([128, kt, 128], F32R, tag="lt")
                            lhsT_load(m, lt)
                            xo = xop.tile([128, G], F32, tag="xo")
                            for n in range(6):
                                ns = slice(n * 512, (n + 1) * 512)
                                ps = pp.tile([mr, 512], F32, tag="pp")
                                for k in range(kt):
                                    nc.tensor.matmul(
                                        ps[:], lt[:, k, :mr], wx_sb[:, k, ns],
                                        start=(k == 0), stop=(k == kt - 1))
                                nc.vector.tensor_tensor(
                                    xo[:mr, ns], ps[:], bb[:mr, ns],
                                    mybir.AluOpType.add)
                            nc.sync.dma_start(
                                xw_dram[d, m * 128:m * 128 + mr, :], xo[:mr])

            def load_from_xfT(m, lt):
                mr = mrows(m)
                for k in range(6):
                    nc.sync.dma_start(
                        lt[:, k, :mr],
                        xfT_d[k * 128:(k + 1) * 128,
                              m * 128:m * 128 + mr].bitcast(F32R))

            def load_from_ysT(m, lt):
                mr = mrows(m)
                for k in range(12):
                    nc.sync.dma_start(
                        lt[:, k, :mr],
                        ysT_d[k * 128:(k + 1) * 128, m * 128:m * 128 + mr])

            # ---------- recurrence (both directions interleaved) ----------
            def recur_phase(wh_dram, xw_dram, ys_dram):
                with tc.tile_pool(name="whp", bufs=1) as whp, \
                     tc.tile_pool(name="st", bufs=1) as stp, \
                     tc.tile_pool(name="pgp", bufs=3) as pgp, \
                     tc.tile_pool(name="gps", bufs=6, space="PSUM") as gpsp, \
                     tc.tile_pool(name="tps", bufs=2, space="PSUM") as tpsp:
                    wh_sb = whp.tile([128, 2, 6, G], F32R)
                    for d in range(2):
                        for k in range(6):
                            nc.sync.dma_start(
                                wh_sb[:, d, k],
                                wh_dram[d, k * 128:(k + 1) * 128, :].bitcast(F32R))
                    h = [stp.tile([SEQ_PC, H], F32, name=f"h{d}") for d in range(2)]
                    c = [stp.tile([SEQ_PC, H], F32, name=f"c{d}") for d in range(2)]
                    tcs = [stp.tile([SEQ_PC, H], F32, name=f"tc{d}") for d in range(2)]
                    tmp = [stp.tile([SEQ_PC, H], F32, name=f"tm{d}") for d in range(2)]
                    hT = [stp.tile([128, 6, SEQ_PC], F32R, name=f"hT{d}")
                          for d in range(2)]
                    for d in range(2):
                        nc.vector.memset(c[d][:], 0.0)

                    xw_r = xw_dram.rearrange("d (s t) g -> d s t g", t=WIDTH)
                    ys_r = ys_dram.rearrange("(s t) g -> s t g", t=WIDTH)

                    abl = os.environ.get("BLSTM_ABLATE", "")
                    for t in range(WIDTH):
                        for d in range(2):
                            tt = t if d == 0 else WIDTH - 1 - t
                            pg = pgp.tile([SEQ_PC, G], F32, tag="pg")
                            nc.sync.dma_start(pg[:], xw_r[d, :, tt, :])
                            if t > 0 and "nomm" not in abl:
                                for n in range(6):
                                    ns = slice(n * 512, (n + 1) * 512)
                                    ps = gpsp.tile([SEQ_PC, 512], F32, tag="g")
                                    for k in range(6):
                                        nc.tensor.matmul(
                                            ps[:], hT[d][:, k], wh_sb[:, d, k, ns],
                                            start=(k == 0), stop=(k == 5))
                                    nc.vector.tensor_tensor(
                                        pg[:, ns], ps[:], pg[:, ns],
                                        mybir.AluOpType.add)
                            # piecewise ACT: each span fires as soon as the
                            # psum-tile adds covering it are done
                            nc.scalar.activation(
                                pg[:, 0:1024], pg[:, 0:1024],
                                mybir.ActivationFunctionType.Sigmoid)
                            nc.scalar.activation(
                                pg[:, 1024:2304], pg[:, 1024:2304],
                                mybir.ActivationFunctionType.Sigmoid)
                            nc.scalar.activation(
                                pg[:, 2304:3072], pg[:, 2304:3072],
                                mybir.ActivationFunctionType.Tanh)
                            # c = f*c + i*g ; h = o*tanh(c)
                            nc.vector.tensor_tensor(
                                tmp[d][:], pg[:, 0:768], pg[:, 2304:3072],
                                mybir.AluOpType.mult)
                            nc.vector.tensor_tensor(
                                c[d][:], c[d][:], pg[:, 768:1536],
                                mybir.AluOpType.mult)
                            nc.vector.tensor_tensor(
                                c[d][:], c[d][:], tmp[d][:], mybir.AluOpType.add)
                            nc.scalar.activation(
                                tcs[d][:], c[d][:],
                                mybir.ActivationFunctionType.Tanh)
                            nc.vector.tensor_tensor(
                                h[d][:], pg[:, 1536:2304], tcs[d][:],
                                mybir.AluOpType.mult)
                            if t < WIDTH - 1 and "notr" not in abl:
                                pt = tpsp.tile([128, 6 * SEQ_PC], F32, tag="t")
                                for k in range(6):
                                    nc.tensor.transpose(
                                        pt[:, k * SEQ_PC:(k + 1) * SEQ_PC],
                                        h[d][:, k * 128:(k + 1) * 128],
                                        ident[:SEQ_PC, :SEQ_PC])
                                nc.vector.tensor_copy(
                                    hT[d].rearrange("p k s -> p (k s)"), pt[:])
                            nc.sync.dma_start(
                                ys_r[:, tt, d * H:(d + 1) * H], h[d][:])

            # ---------- transpose ys -> ysT (f32r) ----------
            def transpose_phase(ys_dram):
                with tc.tile_pool(name="ti", bufs=2) as tip, \
                     tc.tile_pool(name="to", bufs=2) as top, \
                     tc.tile_pool(name="tp", bufs=4, space="PSUM") as tpp:
                    for m in range(MT):
                        mr = mrows(m)
                        ti = tip.tile([128, 2 * H], F32, tag="ti")
                        nc.sync.dma_start(
                            ti[:mr], ys_dram[m * 128:m * 128 + mr, :])
                        for k in range(12):
                            ps = tpp.tile([128, 128], F32, tag="tp")
                            nc.tensor.transpose(
                                ps[:, :mr], ti[:mr, k * 128:(k + 1) * 128],
                                ident[:mr, :mr])
                            to = top.tile([128, 128], F32R, tag="to")
                            nc.vector.tensor_copy(to[:, :mr], ps[:, :mr])
                            nc.sync.dma_start(
                                ysT_d[k * 128:(k + 1) * 128,
                                      m * 128:m * 128 + mr], to[:, :mr])

            # ---------- final linear ----------
            def linear_phase():
                with tc.tile_pool(name="lwp", bufs=1) as lwp, \
                     tc.tile_pool(name="lbp", bufs=1) as lbp, \
                     tc.tile_pool(name="lpp", bufs=4, space="PSUM") as lpp, \
                     tc.tile_pool(name="llt", bufs=2) as lltp, \
                     tc.tile_pool(name="lo", bufs=2) as lop:
                    lw = lwp.tile([128, 12, DIM], F32R)
                    for k in range(12):
                        nc.sync.dma_start(
                            lw[:, k],
                            linw_d[k * 128:(k + 1) * 128, :].bitcast(F32R))
                    lbsb = lbp.tile([1, DIM], F32)
                    nc.sync.dma_start(lbsb[:], linb_d[None, :])
                    lbb = lbp.tile([128, DIM], F32)
                    for n in range(2):
                        ns = slice(n * 384, (n + 1) * 384)
                        bps = lpp.tile([128, 384], F32, tag="lp")
                        nc.tensor.matmul(bps[:], ones[:], lbsb[:, ns],
                                         start=True, stop=True)
                        nc.vector.tensor_copy(lbb[:, ns], bps[:])
                    for m in range(MT):
                        mr = mrows(m)
                        lt = lltp.tile([128, 12, 128], F32R, tag="lt")
                        load_from_ysT(m, lt)
                        lo = lop.tile([128, DIM], F32, tag="lo")
                        for n in range(2):
                            ns = slice(n * 384, (n + 1) * 384)
                            ps = lpp.tile([mr, 384], F32, tag="lp")
                            for k in range(12):
                                nc.tensor.matmul(
                                    ps[:], lt[:, k, :mr], lw[:, k, ns],
                                    start=(k == 0), stop=(k == 11))
                            nc.vector.tensor_tensor(
                                lo[:mr, ns], ps[:], lbb[:mr, ns],
                                mybir.AluOpType.add)
                        nc.sync.dma_start(out_d[m * 128:m * 128 + mr, :], lo[:mr])

            proj_phase(6, load_from_xfT, wx0_d, b0_d, xw0_d)
            recur_phase(wh0_d, xw0_d, ys0_d)
            transpose_phase(ys0_d)
            proj_phase(12, load_from_ysT, wx1_d, b1_d, xw1_d)
            recur_phase(wh1_d, xw1_d, ys1_d)
            transpose_phase(ys1_d)
            linear_phase()

    nc.compile()
    return nc


def _reorder_gates(w):
    """[i f g o] -> [i f o g] along last axis (size 4H)."""
    i, f, g, o = np.split(w, 4, axis=-1)
    return np.concatenate([i, f, o, g], axis=-1)


def kernel(x, Wx0f, Wh0f, b0f, Wx0b, Wh0b, b0b,
           Wx1f, Wh1f, b1f, Wx1b, Wh1b, b1b, lin_W, lin_b):
    x = np.asarray(x, dtype=np.float32)
    # frame: (B, C, T) -> (NSEQ, WIDTH, C)
    tgt = (NFR - 1) * STRIDE + WIDTH
    xp = np.zeros((B, DIM, tgt), dtype=np.float32)
    xp[:, :, :T] = x
    frames = np.stack([xp[:, :, i:i + WIDTH]
                       for i in range(0, tgt - WIDTH + 1, STRIDE)], axis=1)
    xf = frames.reshape(NSEQ, DIM, WIDTH).transpose(0, 2, 1)  # (80, 200, 768)

    def prep(wf, wb):
        return np.ascontiguousarray(
            np.stack([_reorder_gates(np.asarray(wf, np.float32)),
                      _reorder_gates(np.asarray(wb, np.float32))]))

    wx0 = prep(Wx0f, Wx0b)
    wh0 = prep(Wh0f, Wh0b)
    b0 = prep(b0f, b0b)
    wx1 = prep(Wx1f, Wx1b)
    wh1 = prep(Wh1f, Wh1b)
    b1 = prep(b1f, b1b)
    linw = np.ascontiguousarray(np.asarray(lin_W, np.float32))
    linb = np.ascontiguousarray(np.asarray(lin_b, np.float32))

    if "nc" not in _CACHE:
        _CACHE["nc"] = _build_program()
    nc = _CACHE["nc"]

    in_maps = []
    for cc in range(NCORES):
        shard = xf[cc * SEQ_PC:(cc + 1) * SEQ_PC]          # (10, 200, 768)
        xfT = np.ascontiguousarray(shard.reshape(ROWS, DIM).T)  # (768, 2000)
        in_maps.append({"xfT": xfT, "wx0": wx0, "wh0": wh0, "b0": b0,
                        "wx1": wx1, "wh1": wh1, "b1": b1,
                        "linw": linw, "linb": linb})
    _CACHE["in_maps"] = in_maps

    res = run_bass_kernel_spmd(nc, in_maps, list(range(NCORES)))
    outs = [res.results[cc]["out"].reshape(SEQ_PC, WIDTH, DIM)
            for cc in range(NCORES)]
    y = np.concatenate(outs, axis=0)                        # (80, 200, 768)
    y = y.transpose(0, 2, 1).reshape(B, NFR, DIM, WIDTH)    # (4,20,768,200)

    limit = STRIDE // 2
    parts = [y[:, 0, :, :-limit]]
    for k in range(1, NFR - 1):
        parts.append(y[:, k, :, limit:-limit])
    parts.append(y[:, NFR - 1, :, limit:])
    yc = np.concatenate(parts, axis=-1)[:, :, :T]           # (4, 768, 2000)
    return (yc + x).astype(np.float32)



# revision 15
# speedup vs baseline: 3.5760x; 3.5760x over previous
"""Bass/Trainium2 kernel for framed 2-layer BiLSTM (nn_BLSTM).

Data-parallel over the 80 framed sequences: 10 per core on 8 NeuronCores.
All matmuls in bf16 (f32 PSUM accumulation). The recurrence runs in a
TRANSPOSED layout [channels(partitions) x sequences(free)]: per step the
xw slice is injected into PSUM via PE transposes (start of the accumulate
group) and the Wh contribution streams as 128x128-stationary matmuls with
N=10 moving columns. Hidden states land directly in resident transposed
h-buffers that feed both the next step's matmuls and the next layer's
input projection as stationary operands.
"""
import sys
import numpy as np

sys.path.insert(0, "/opt/trn_rl_repo")

import ml_dtypes  # noqa: E402
import concourse.bass as bass  # noqa: E402
import concourse.mybir as mybir  # noqa: E402
from concourse import bacc  # noqa: E402
from concourse.tile import TileContext  # noqa: E402
from concourse.masks import make_identity  # noqa: E402
from concourse.bass_utils import run_bass_kernel_spmd  # noqa: E402

F32 = mybir.dt.float32
BF16 = mybir.dt.bfloat16
BF16_NP = ml_dtypes.bfloat16

DIM = 768
H = 768
G = 4 * H            # 3072, gate order reordered to [i, f, o, g]
B, T = 4, 2000
WIDTH, STRIDE = 200, 100
NFR = 20             # frames per batch element
NSEQ = B * NFR       # 80
NCORES = 8
SEQ_PC = NSEQ // NCORES   # 10
ROWS = SEQ_PC * WIDTH     # 2000 rows per core, row = t*10 + s (t-major)
MT = (ROWS + 127) // 128  # 16 row m-tiles (last has 80 rows)
CH_STEPS = 10             # timesteps per xw chunk DMA
NCH = WIDTH // CH_STEPS   # 20 chunks

SIG = mybir.ActivationFunctionType.Sigmoid
TANH = mybir.ActivationFunctionType.Tanh
MUL = mybir.AluOpType.mult
ADD = mybir.AluOpType.add

_CACHE = {}


def _mrows(m):
    return min(128, ROWS - m * 128)


def _build_program():
    nc = bacc.Bacc("TRN2", target_bir_lowering=False, debug=False,
                   num_devices=NCORES)

    xT_d = nc.declare_dram_parameter("xT", [DIM, ROWS], BF16, isOutput=False)
    wx0_d = nc.declare_dram_parameter("wx0", [2, DIM, G], BF16, isOutput=False)
    wh0_d = nc.declare_dram_parameter("wh0", [2, H, G], BF16, isOutput=False)
    b0_d = nc.declare_dram_parameter("b0", [2, 128, 24], F32, isOutput=False)
    wx1_d = nc.declare_dram_parameter("wx1", [2, 2 * H, G], BF16,
                                      isOutput=False)
    wh1_d = nc.declare_dram_parameter("wh1", [2, H, G], BF16, isOutput=False)
    b1_d = nc.declare_dram_parameter("b1", [2, 128, 24], F32, isOutput=False)
    linw_d = nc.declare_dram_parameter("linw", [2 * H, DIM], BF16,
                                       isOutput=False)
    linb_d = nc.declare_dram_parameter("linb", [DIM], F32, isOutput=False)
    out_d = nc.declare_dram_parameter("out", [ROWS, DIM], F32, isOutput=True)

    # xw stored transposed: [dir, gate m-tile, partition(gate%128), row]
    xw0_d = nc.dram_tensor("xw0", [2, 24, 128, ROWS], BF16)
    xw1_d = nc.dram_tensor("xw1", [2, 24, 128, ROWS], BF16)

    with TileContext(nc) as tc:
        with tc.tile_pool(name="const", bufs=1) as constp:
            identb = constp.tile([128, 128], BF16)
            make_identity(nc, identb[:])
            ones = constp.tile([1, 128], F32)
            nc.vector.memset(ones[:], 1.0)

            # ---- batched input projection, transposed output ----
            # xwT[d, m, p, row] = sum_k rhs_fn(row)[k] * Wx[k, m*128+p] + b
            def proj(kt, rhs_fn, wx_dram, b_dram, xw_dram):
                for d in range(2):
                    with tc.tile_pool(name="wxp", bufs=1) as wxp, \
                         tc.tile_pool(name="bbp", bufs=1) as bbp, \
                         tc.tile_pool(name="pp", bufs=3, space="PSUM") as pp, \
                         tc.tile_pool(name="xo", bufs=3) as xop:
                        wx_sb = wxp.tile([128, kt, G], BF16)
                        for k in range(kt):
                            nc.sync.dma_start(
                                wx_sb[:, k],
                                wx_dram[d, k * 128:(k + 1) * 128, :])
                        bT = bbp.tile([128, 24], F32)
                        nc.sync.dma_start(bT[:], b_dram[d])
                        for m in range(24):
                            for r in range(4):
                                rs = slice(r * 500, (r + 1) * 500)
                                ps = pp.tile([128, 500], F32, tag="pp")
                                for k in range(kt):
                                    nc.tensor.matmul(
                                        ps[:],
                                        wx_sb[:, k, m * 128:(m + 1) * 128],
                                        rhs_fn(d, k, rs),
                                        start=(k == 0), stop=(k == kt - 1))
                                xo = xop.tile([128, 500], BF16, tag="xo")
                                nc.vector.tensor_scalar(
                                    xo[:], ps[:], bT[:, m:m + 1], None, ADD)
                                nc.sync.dma_start(xw_dram[d, m, :, rs], xo[:])

            # ---- recurrence, both dirs interleaved, transposed layout ----
            def recur(wh_dram, xw_dram, hT):
                with tc.tile_pool(name="whp", bufs=1) as whp, \
                     tc.tile_pool(name="stp", bufs=1) as stp, \
                     tc.tile_pool(name="xcp", bufs=2) as xcp, \
                     tc.tile_pool(name="gpp", bufs=2, space="PSUM") as gpp, \
                     tc.tile_pool(name="sgp", bufs=2) as sgp:
                    wh_sb = whp.tile([128, 2, 6, G], BF16)
                    for d in range(2):
                        for k in range(6):
                            nc.sync.dma_start(
                                wh_sb[:, d, k],
                                wh_dram[d, k * 128:(k + 1) * 128, :])
                    c = [stp.tile([128, 60], F32, name=f"c{d}")
                         for d in range(2)]
                    for d in range(2):
                        nc.vector.memset(c[d][:], 0.0)

                    for ch in range(NCH):
                        xc = []
                        for d in range(2):
                            cs = ch * 100 if d == 0 else 1900 - ch * 100
                            xct = xcp.tile([128, 24, 100], BF16,
                                           tag=f"xc{d}", name=f"xc{d}")
                            nc.sync.dma_start(
                                xct[:],
                                xw_dram[d, :, :, cs:cs + 100].rearrange(
                                    "m p c -> p m c"))
                            xc.append(xct)
                        for tl in range(CH_STEPS):
                            t = ch * CH_STEPS + tl
                            first = (t == 0)
                            sg, tg, tm, tcs = [], [], [], []
                            for d in range(2):
                                tt = t if d == 0 else WIDTH - 1 - t
                                lc = tl * 10 if d == 0 else (9 - tl) * 10
                                g = gpp.tile([128, 240], F32, tag=f"g{d}")
                                for m in range(24):
                                    ms = slice(m * 10, (m + 1) * 10)
                                    nc.tensor.matmul(
                                        g[:, ms],
                                        identb[:],
                                        xc[d][:, m, lc:lc + 10],
                                        start=True, stop=first)
                                    if not first:
                                        pv = (t - 1) * 10 if d == 0 \
                                            else (tt + 1) * 10
                                        for k in range(6):
                                            nc.tensor.matmul(
                                                g[:, ms],
                                                wh_sb[:, d, k,
                                                      m * 128:(m + 1) * 128],
                                                hT[d][:, k, pv:pv + 10],
                                                start=False, stop=(k == 5))
                                sg.append(sgp.tile([128, 180], F32,
                                                   tag=f"sg{d}",
                                                   name=f"sg{d}"))
                                tg.append(sgp.tile([128, 60], F32,
                                                   tag=f"tg{d}",
                                                   name=f"tg{d}"))
                                tm.append(sgp.tile([128, 60], F32,
                                                   tag=f"tm{d}",
                                                   name=f"tm{d}"))
                                tcs.append(sgp.tile([128, 60], F32,
                                                    tag=f"tc{d}",
                                                    name=f"tc{d}"))
                                nc.scalar.activation(sg[d][:], g[:, 0:180],
                                                     SIG)
                                nc.scalar.activation(tg[d][:], g[:, 180:240],
                                                     TANH)
                                nc.gpsimd.tensor_tensor(
                                    tm[d][:], sg[d][:, 0:60], tg[d][:], MUL)
                                nc.vector.tensor_tensor(
                                    c[d][:], c[d][:], sg[d][:, 60:120], MUL)
                                nc.vector.tensor_tensor(
                                    c[d][:], c[d][:], tm[d][:], ADD)
                            for d in range(2):
                                tt = t if d == 0 else WIDTH - 1 - t
                                nc.scalar.activation(tcs[d][:], c[d][:], TANH)
                                nc.vector.tensor_tensor(
                                    hT[d][:, :, tt * 10:(tt + 1) * 10],
                                    sg[d][:, 120:180].rearrange(
                                        "p (a b) -> p a b", b=10),
                                    tcs[d][:].rearrange(
                                        "p (a b) -> p a b", b=10),
                                    MUL)

            # ---- final linear: y = h1cat @ linW + linb ----
            def linear(hT1):
                with tc.tile_pool(name="lwp", bufs=1) as lwp, \
                     tc.tile_pool(name="lpp", bufs=3, space="PSUM") as lpp, \
                     tc.tile_pool(name="lop", bufs=2) as lop:
                    lw = lwp.tile([128, 12, DIM], BF16)
                    for k in range(12):
                        nc.sync.dma_start(lw[:, k],
                                          linw_d[k * 128:(k + 1) * 128, :])
                    lbsb = lwp.tile([1, DIM], F32)
                    nc.sync.dma_start(lbsb[:], linb_d[None, :])
                    lbb = lwp.tile([128, DIM], F32)
                    for n in range(2):
                        ns = slice(n * 384, (n + 1) * 384)
                        bps = lpp.tile([128, 384], F32, tag="lp")
                        nc.tensor.matmul(bps[:], ones[:], lbsb[:, ns],
                                         start=True, stop=True)
                        nc.vector.tensor_copy(lbb[:, ns], bps[:])
                    for m in range(MT):
                        mr = _mrows(m)
                        lo = lop.tile([128, DIM], F32, tag="lo")
                        for n in range(2):
                            ns = slice(n * 384, (n + 1) * 384)
                            ps = lpp.tile([mr, 384], F32, tag="lp")
                            for k in range(12):
                                ht = hT1[0] if k < 6 else hT1[1]
                                kk = k if k < 6 else k - 6
                                nc.tensor.matmul(
                                    ps[:],
                                    ht[:, kk, m * 128:m * 128 + mr],
                                    lw[:, k, ns],
                                    start=(k == 0), stop=(k == 11))
                            nc.vector.tensor_tensor(
                                lo[:mr, ns], ps[:], lbb[:mr, ns], ADD)
                        nc.sync.dma_start(out_d[m * 128:m * 128 + mr, :],
                                          lo[:mr])

            with tc.tile_pool(name="hb1", bufs=1) as hb1p:
                hT1 = [hb1p.tile([128, 6, ROWS], BF16, name=f"h1{d}")
                       for d in range(2)]
                with tc.tile_pool(name="hb0", bufs=1) as hb0p:
                    hT0 = [hb0p.tile([128, 6, ROWS], BF16, name=f"h0{d}")
                           for d in range(2)]
                    with tc.tile_pool(name="xtp", bufs=1) as xtp:
                        xT_sb = xtp.tile([128, 6, ROWS], BF16)
                        for k in range(6):
                            nc.sync.dma_start(xT_sb[:, k],
                                              xT_d[k * 128:(k + 1) * 128, :])
                        proj(6,
                             lambda d, k, rs: xT_sb[:, k, rs],
                             wx0_d, b0_d, xw0_d)
                    recur(wh0_d, xw0_d, hT0)
                    proj(12,
                         lambda d, k, rs:
                         hT0[0][:, k, rs] if k < 6
                         else hT0[1][:, k - 6, rs],
                         wx1_d, b1_d, xw1_d)
                recur(wh1_d, xw1_d, hT1)
                linear(hT1)

    nc.compile()
    return nc


def _reorder_gates(w):
    """[i f g o] -> [i f o g] along last axis (size 4H)."""
    i, f, g, o = np.split(w, 4, axis=-1)
    return np.concatenate([i, f, o, g], axis=-1)


def kernel(x, Wx0f, Wh0f, b0f, Wx0b, Wh0b, b0b,
           Wx1f, Wh1f, b1f, Wx1b, Wh1b, b1b, lin_W, lin_b):
    x = np.asarray(x, dtype=np.float32)
    # frame: (B, C, T) -> (NSEQ, DIM, WIDTH)
    tgt = (NFR - 1) * STRIDE + WIDTH
    xp = np.zeros((B, DIM, tgt), dtype=np.float32)
    xp[:, :, :T] = x
    frames = np.stack([xp[:, :, i:i + WIDTH]
                       for i in range(0, tgt - WIDTH + 1, STRIDE)], axis=1)
    xf = frames.reshape(NSEQ, DIM, WIDTH)

    def prepw(wf, wb):
        return np.ascontiguousarray(np.stack(
            [_reorder_gates(np.asarray(wf, np.float32)),
             _reorder_gates(np.asarray(wb, np.float32))])).astype(BF16_NP)

    def prepb(bf, bb_):
        # transposed bias: [dir, partition(gate%128), gate m-tile]
        return np.ascontiguousarray(np.stack(
            [_reorder_gates(np.asarray(bf, np.float32)).reshape(24, 128).T,
             _reorder_gates(np.asarray(bb_, np.float32)).reshape(24, 128).T]))

    wx0 = prepw(Wx0f, Wx0b)
    wh0 = prepw(Wh0f, Wh0b)
    b0 = prepb(b0f, b0b)
    wx1 = prepw(Wx1f, Wx1b)
    wh1 = prepw(Wh1f, Wh1b)
    b1 = prepb(b1f, b1b)
    linw = np.ascontiguousarray(np.asarray(lin_W, np.float32)).astype(BF16_NP)
    linb = np.ascontiguousarray(np.asarray(lin_b, np.float32))

    if "nc" not in _CACHE:
        _CACHE["nc"] = _build_program()
    nc = _CACHE["nc"]

    in_maps = []
    for cc in range(NCORES):
        shard = xf[cc * SEQ_PC:(cc + 1) * SEQ_PC]       # (10, 768, 200)
        xT = shard.transpose(1, 2, 0).reshape(DIM, ROWS)  # col = t*10 + s
        in_maps.append({"xT": np.ascontiguousarray(xT).astype(BF16_NP),
                        "wx0": wx0, "wh0": wh0, "b0": b0,
                        "wx1": wx1, "wh1": wh1, "b1": b1,
                        "linw": linw, "linb": linb})
    _CACHE["in_maps"] = in_maps

    res = run_bass_kernel_spmd(nc, in_maps, list(range(NCORES)))
    outs = [np.asarray(res.results[cc]["out"], np.float32)
            .reshape(WIDTH, SEQ_PC, DIM).transpose(1, 0, 2)
            for cc in range(NCORES)]                     # (10, 200, 768)
    y = np.concatenate(outs, axis=0)                     # (80, 200, 768)
    y = y.transpose(0, 2, 1).reshape(B, NFR, DIM, WIDTH)

    limit = STRIDE // 2
    parts = [y[:, 0, :, :-limit]]
    for k in range(1, NFR - 1):
        parts.append(y[:, k, :, limit:-limit])
    parts.append(y[:, NFR - 1, :, limit:])
    yc = np.concatenate(parts, axis=-1)[:, :, :T]        # (4, 768, 2000)
    return (yc + x).astype(np.float32)


# revision 19
# speedup vs baseline: 4.1046x; 1.1478x over previous
"""Bass/Trainium2 kernel for framed 2-layer BiLSTM (nn_BLSTM).

Data-parallel over the 80 framed sequences: 10 per core on 8 NeuronCores.
All matmuls in bf16 (f32 PSUM accumulation). The recurrence runs in a
TRANSPOSED layout [channels(partitions) x sequences(free)]: per step the
xw slice is injected into PSUM via PE transposes (start of the accumulate
group) and the Wh contribution streams as 128x128-stationary matmuls with
N=10 moving columns. Hidden states land directly in resident transposed
h-buffers that feed both the next step's matmuls and the next layer's
input projection as stationary operands.
"""
import sys
import numpy as np

sys.path.insert(0, "/opt/trn_rl_repo")

import ml_dtypes  # noqa: E402
import concourse.bass as bass  # noqa: E402
import concourse.mybir as mybir  # noqa: E402
from concourse import bacc  # noqa: E402
from concourse.tile import TileContext  # noqa: E402
from concourse.masks import make_identity  # noqa: E402
from concourse.bass_utils import run_bass_kernel_spmd  # noqa: E402

F32 = mybir.dt.float32
BF16 = mybir.dt.bfloat16
BF16_NP = ml_dtypes.bfloat16

DIM = 768
H = 768
G = 4 * H            # 3072, gate order reordered to [i, f, o, g]
B, T = 4, 2000
WIDTH, STRIDE = 200, 100
NFR = 20             # frames per batch element
NSEQ = B * NFR       # 80
NCORES = 8
SEQ_PC = NSEQ // NCORES   # 10
ROWS = SEQ_PC * WIDTH     # 2000 rows per core, row = t*10 + s (t-major)
MT = (ROWS + 127) // 128  # 16 row m-tiles (last has 80 rows)
CH_STEPS = 10             # timesteps per xw chunk DMA
NCH = WIDTH // CH_STEPS   # 20 chunks

SIG = mybir.ActivationFunctionType.Sigmoid
TANH = mybir.ActivationFunctionType.Tanh
MUL = mybir.AluOpType.mult
ADD = mybir.AluOpType.add

_CACHE = {}


def _mrows(m):
    return min(128, ROWS - m * 128)


def _build_program():
    nc = bacc.Bacc("TRN2", target_bir_lowering=False, debug=False,
                   num_devices=NCORES)

    xT_d = nc.declare_dram_parameter("xT", [DIM, ROWS], BF16, isOutput=False)
    wx0_d = nc.declare_dram_parameter("wx0", [2, DIM, G], BF16, isOutput=False)
    wh0_d = nc.declare_dram_parameter("wh0", [2, H, G], BF16, isOutput=False)
    b0_d = nc.declare_dram_parameter("b0", [2, 128, 24], F32, isOutput=False)
    wx1_d = nc.declare_dram_parameter("wx1", [2, 2 * H, G], BF16,
                                      isOutput=False)
    wh1_d = nc.declare_dram_parameter("wh1", [2, H, G], BF16, isOutput=False)
    b1_d = nc.declare_dram_parameter("b1", [2, 128, 24], F32, isOutput=False)
    linw_d = nc.declare_dram_parameter("linw", [2 * H, DIM], BF16,
                                       isOutput=False)
    linb_d = nc.declare_dram_parameter("linb", [DIM], F32, isOutput=False)
    out_d = nc.declare_dram_parameter("out", [ROWS, DIM], F32, isOutput=True)

    # xw stored transposed: [dir, gate m-tile, partition(gate%128), row]
    xw0_d = nc.dram_tensor("xw0", [2, 24, 128, ROWS], BF16)
    xw1_d = nc.dram_tensor("xw1", [2, 24, 128, ROWS], BF16)

    with TileContext(nc) as tc:
        with tc.tile_pool(name="const", bufs=1) as constp:
            identb = constp.tile([128, 128], BF16)
            make_identity(nc, identb[:])
            ones = constp.tile([1, 128], F32)
            nc.vector.memset(ones[:], 1.0)

            # ---- batched input projection, transposed output ----
            # xwT[d, m, p, row] = sum_k rhs_fn(row)[k] * Wx[k, m*128+p] + b
            def proj(kt, rhs_fn, wx_dram, b_dram, xw_dram, mid=None):
                for d in range(2):
                    if d == 1 and mid is not None:
                        mid()
                    with tc.tile_pool(name="wxp", bufs=1) as wxp, \
                         tc.tile_pool(name="bbp", bufs=1) as bbp, \
                         tc.tile_pool(name="pp", bufs=3, space="PSUM") as pp, \
                         tc.tile_pool(name="xo", bufs=3) as xop:
                        wx_sb = wxp.tile([128, kt, G], BF16)
                        for k in range(kt):
                            nc.sync.dma_start(
                                wx_sb[:, k],
                                wx_dram[d, k * 128:(k + 1) * 128, :])
                        bT = bbp.tile([128, 24], F32)
                        nc.sync.dma_start(bT[:], b_dram[d])
                        for m in range(24):
                            for r in range(4):
                                rs = slice(r * 500, (r + 1) * 500)
                                ps = pp.tile([128, 500], F32, tag="pp")
                                for k in range(kt):
                                    nc.tensor.matmul(
                                        ps[:],
                                        wx_sb[:, k, m * 128:(m + 1) * 128],
                                        rhs_fn(d, k, rs),
                                        start=(k == 0), stop=(k == kt - 1))
                                xo = xop.tile([128, 500], BF16, tag="xo")
                                nc.vector.tensor_scalar(
                                    xo[:], ps[:], bT[:, m:m + 1], None, ADD)
                                nc.sync.dma_start(xw_dram[d, m, :, rs], xo[:])

            # ---- recurrence, both dirs interleaved, transposed layout ----
            # Gates split into two PSUM halves: [i,f] (m-tiles 0-11) and
            # [o,g] (m-tiles 12-23), so the sigmoid chain of a step starts
            # after only half its matmuls and hides under the rest.
            def recur(whp, wh_sb, xw_dram, hT):
                with tc.tile_pool(name="stp", bufs=1) as stp, \
                     tc.tile_pool(name="xcp", bufs=2) as xcp, \
                     tc.tile_pool(name="gpp", bufs=2, space="PSUM") as gpp, \
                     tc.tile_pool(name="sgp", bufs=2) as sgp:
                    c = [stp.tile([128, 60], F32, name=f"c{d}")
                         for d in range(2)]
                    for d in range(2):
                        nc.vector.memset(c[d][:], 0.0)

                    for ch in range(NCH):
                        xc = []
                        for d in range(2):
                            cs = ch * 100 if d == 0 else 1900 - ch * 100
                            xct = xcp.tile([128, 24, 100], BF16,
                                           tag=f"xc{d}", name=f"xc{d}")
                            nc.sync.dma_start(
                                xct[:],
                                xw_dram[d, :, :, cs:cs + 100].rearrange(
                                    "m p c -> p m c"))
                            xc.append(xct)
                        for tl in range(CH_STEPS):
                            t = ch * CH_STEPS + tl
                            first = (t == 0)
                            sg, tcs = [], []
                            for d in range(2):
                                tt = t if d == 0 else WIDTH - 1 - t
                                lc = tl * 10 if d == 0 else (9 - tl) * 10
                                pv = (t - 1) * 10 if d == 0 else (tt + 1) * 10
                                gif = gpp.tile([128, 120], F32,
                                               tag=f"gi{d}", name=f"gi{d}")
                                gog = gpp.tile([128, 120], F32,
                                               tag=f"go{d}", name=f"go{d}")
                                for m in range(24):
                                    gg = gif if m < 12 else gog
                                    ms = slice((m % 12) * 10,
                                               (m % 12) * 10 + 10)
                                    nc.tensor.matmul(
                                        gg[:, ms],
                                        identb[:],
                                        xc[d][:, m, lc:lc + 10],
                                        start=True, stop=first)
                                    if not first:
                                        for k in range(6):
                                            nc.tensor.matmul(
                                                gg[:, ms],
                                                wh_sb[:, d, k,
                                                      m * 128:(m + 1) * 128],
                                                hT[d][:, k, pv:pv + 10],
                                                start=False, stop=(k == 5))
                                sgif = sgp.tile([128, 120], F32,
                                                tag=f"si{d}", name=f"si{d}")
                                tgg = sgp.tile([128, 60], F32,
                                               tag=f"tg{d}", name=f"tg{d}")
                                sgo = sgp.tile([128, 60], F32,
                                               tag=f"so{d}", name=f"so{d}")
                                tmm = sgp.tile([128, 60], F32,
                                               tag=f"tm{d}", name=f"tm{d}")
                                sg.append((sgif, sgo))
                                tcs.append(sgp.tile([128, 60], F32,
                                                    tag=f"tc{d}",
                                                    name=f"tc{d}"))
                                nc.scalar.activation(sgif[:], gif[:], SIG)
                                nc.scalar.activation(tgg[:], gog[:, 60:120],
                                                     TANH)
                                nc.scalar.activation(sgo[:], gog[:, 0:60],
                                                     SIG)
                                nc.gpsimd.tensor_tensor(
                                    tmm[:], sgif[:, 0:60], tgg[:], MUL)
                                nc.vector.tensor_tensor(
                                    c[d][:], c[d][:], sgif[:, 60:120], MUL)
                                nc.vector.tensor_tensor(
                                    c[d][:], c[d][:], tmm[:], ADD)
                            for d in range(2):
                                tt = t if d == 0 else WIDTH - 1 - t
                                nc.scalar.activation(tcs[d][:], c[d][:], TANH)
                                nc.vector.tensor_tensor(
                                    hT[d][:, :, tt * 10:(tt + 1) * 10],
                                    sg[d][1][:].rearrange(
                                        "p (a b) -> p a b", b=10),
                                    tcs[d][:].rearrange(
                                        "p (a b) -> p a b", b=10),
                                    MUL)

            # ---- final linear: y = h1cat @ linW + linb ----
            def linear(hT1, lwp, lw, lbsb):
                with tc.tile_pool(name="lpp", bufs=3, space="PSUM") as lpp, \
                     tc.tile_pool(name="lop", bufs=2) as lop:
                    lbb = lwp.tile([128, DIM], F32)
                    for n in range(2):
                        ns = slice(n * 384, (n + 1) * 384)
                        bps = lpp.tile([128, 384], F32, tag="lp")
                        nc.tensor.matmul(bps[:], ones[:], lbsb[:, ns],
                                         start=True, stop=True)
                        nc.vector.tensor_copy(lbb[:, ns], bps[:])
                    for m in range(MT):
                        mr = _mrows(m)
                        lo = lop.tile([128, DIM], F32, tag="lo")
                        for n in range(2):
                            ns = slice(n * 384, (n + 1) * 384)
                            ps = lpp.tile([mr, 384], F32, tag="lp")
                            for k in range(12):
                                ht = hT1[0] if k < 6 else hT1[1]
                                kk = k if k < 6 else k - 6
                                nc.tensor.matmul(
                                    ps[:],
                                    ht[:, kk, m * 128:m * 128 + mr],
                                    lw[:, k, ns],
                                    start=(k == 0), stop=(k == 11))
                            nc.vector.tensor_tensor(
                                lo[:mr, ns], ps[:], lbb[:mr, ns], ADD)
                        nc.sync.dma_start(out_d[m * 128:m * 128 + mr, :],
                                          lo[:mr])

            def load_wh(whp, wh_dram, name):
                wh_sb = whp.tile([128, 2, 6, G], BF16, name=name)
                for d in range(2):
                    for k in range(6):
                        nc.sync.dma_start(
                            wh_sb[:, d, k],
                            wh_dram[d, k * 128:(k + 1) * 128, :])
                return wh_sb

            # h0/h1 share 2 slots: h1 reuses h0's space after proj1.
            with tc.tile_pool(name="hbp", bufs=2) as hbp:
                hT0 = [hbp.tile([128, 6, ROWS], BF16, tag="hb",
                                name=f"h0{d}") for d in range(2)]
                hT1 = [hbp.tile([128, 6, ROWS], BF16, tag="hb",
                                name=f"h1{d}") for d in range(2)]
                with tc.tile_pool(name="whp0", bufs=1) as whp0:
                    wh0_box = []
                    with tc.tile_pool(name="xtp", bufs=1) as xtp:
                        xT_sb = xtp.tile([128, 6, ROWS], BF16)
                        for k in range(6):
                            nc.sync.dma_start(xT_sb[:, k],
                                              xT_d[k * 128:(k + 1) * 128, :])
                        proj(6,
                             lambda d, k, rs: xT_sb[:, k, rs],
                             wx0_d, b0_d, xw0_d,
                             mid=lambda: wh0_box.append(
                                 load_wh(whp0, wh0_d, "wh0")))
                    recur(whp0, wh0_box[0], xw0_d, hT0)
                with tc.tile_pool(name="whp1", bufs=1) as whp1:
                    wh1_box = []
                    proj(12,
                         lambda d, k, rs:
                         hT0[0][:, k, rs] if k < 6
                         else hT0[1][:, k - 6, rs],
                         wx1_d, b1_d, xw1_d,
                         mid=lambda: wh1_box.append(
                             load_wh(whp1, wh1_d, "wh1")))
                    with tc.tile_pool(name="lwp", bufs=1) as lwp:
                        lw = lwp.tile([128, 12, DIM], BF16)
                        for k in range(12):
                            nc.sync.dma_start(
                                lw[:, k], linw_d[k * 128:(k + 1) * 128, :])
                        lbsb = lwp.tile([1, DIM], F32)
                        nc.sync.dma_start(lbsb[:], linb_d[None, :])
                        recur(whp1, wh1_box[0], xw1_d, hT1)
                        linear(hT1, lwp, lw, lbsb)

    nc.compile()
    return nc


def _reorder_gates(w):
    """[i f g o] -> [i f o g] along last axis (size 4H)."""
    i, f, g, o = np.split(w, 4, axis=-1)
    return np.concatenate([i, f, o, g], axis=-1)


def kernel(x, Wx0f, Wh0f, b0f, Wx0b, Wh0b, b0b,
           Wx1f, Wh1f, b1f, Wx1b, Wh1b, b1b, lin_W, lin_b):
    x = np.asarray(x, dtype=np.float32)
    # frame: (B, C, T) -> (NSEQ, DIM, WIDTH)
    tgt = (NFR - 1) * STRIDE + WIDTH
    xp = np.zeros((B, DIM, tgt), dtype=np.float32)
    xp[:, :, :T] = x
    frames = np.stack([xp[:, :, i:i + WIDTH]
                       for i in range(0, tgt - WIDTH + 1, STRIDE)], axis=1)
    xf = frames.reshape(NSEQ, DIM, WIDTH)

    def prepw(wf, wb):
        return np.ascontiguousarray(np.stack(
            [_reorder_gates(np.asarray(wf, np.float32)),
             _reorder_gates(np.asarray(wb, np.float32))])).astype(BF16_NP)

    def prepb(bf, bb_):
        # transposed bias: [dir, partition(gate%128), gate m-tile]
        return np.ascontiguousarray(np.stack(
            [_reorder_gates(np.asarray(bf, np.float32)).reshape(24, 128).T,
             _reorder_gates(np.asarray(bb_, np.float32)).reshape(24, 128).T]))

    wx0 = prepw(Wx0f, Wx0b)
    wh0 = prepw(Wh0f, Wh0b)
    b0 = prepb(b0f, b0b)
    wx1 = prepw(Wx1f, Wx1b)
    wh1 = prepw(Wh1f, Wh1b)
    b1 = prepb(b1f, b1b)
    linw = np.ascontiguousarray(np.asarray(lin_W, np.float32)).astype(BF16_NP)
    linb = np.ascontiguousarray(np.asarray(lin_b, np.float32))

    if "nc" not in _CACHE:
        _CACHE["nc"] = _build_program()
    nc = _CACHE["nc"]

    in_maps = []
    for cc in range(NCORES):
        shard = xf[cc * SEQ_PC:(cc + 1) * SEQ_PC]       # (10, 768, 200)
        xT = shard.transpose(1, 2, 0).reshape(DIM, ROWS)  # col = t*10 + s
        in_maps.append({"xT": np.ascontiguousarray(xT).astype(BF16_NP),
                        "wx0": wx0, "wh0": wh0, "b0": b0,
                        "wx1": wx1, "wh1": wh1, "b1": b1,
                        "linw": linw, "linb": linb})
    _CACHE["in_maps"] = in_maps

    res = run_bass_kernel_spmd(nc, in_maps, list(range(NCORES)))
    outs = [np.asarray(res.results[cc]["out"], np.float32)
            .reshape(WIDTH, SEQ_PC, DIM).transpose(1, 0, 2)
            for cc in range(NCORES)]                     # (10, 200, 768)
    y = np.concatenate(outs, axis=0)                     # (80, 200, 768)
    y = y.transpose(0, 2, 1).reshape(B, NFR, DIM, WIDTH)

    limit = STRIDE // 2
    parts = [y[:, 0, :, :-limit]]
    for k in range(1, NFR - 1):
        parts.append(y[:, k, :, limit:-limit])
    parts.append(y[:, NFR - 1, :, limit:])
    yc = np.concatenate(parts, axis=-1)[:, :, :T]        # (4, 768, 2000)
    return (yc + x).astype(np.float32)


# revision 29
# speedup vs baseline: 4.8318x; 1.1772x over previous
"""Bass/Trainium2 kernel for framed 2-layer BiLSTM (nn_BLSTM).

Data-parallel over the 80 framed sequences: 10 per core on 8 NeuronCores.
All matmuls in bf16 (f32 PSUM accumulation). The recurrence runs in a
TRANSPOSED layout [channels(partitions) x sequences(free)]: per step the
xw slice is injected into PSUM via PE transposes (start of the accumulate
group) and the Wh contribution streams as 128x128-stationary matmuls with
N=10 moving columns. Hidden states land directly in resident transposed
h-buffers that feed both the next step's matmuls and the next layer's
input projection as stationary operands.
"""
import sys
import numpy as np

sys.path.insert(0, "/opt/trn_rl_repo")

import ml_dtypes  # noqa: E402
import concourse.bass as bass  # noqa: E402
import concourse.mybir as mybir  # noqa: E402
from concourse import bacc  # noqa: E402
from concourse.tile import TileContext  # noqa: E402
from concourse.masks import make_identity  # noqa: E402
from concourse.bass_utils import run_bass_kernel_spmd  # noqa: E402

F32 = mybir.dt.float32
BF16 = mybir.dt.bfloat16
FP8 = mybir.dt.float8e4
BF16_NP = ml_dtypes.bfloat16
FP8_NP = ml_dtypes.float8_e4m3
DR = mybir.MatmulPerfMode.DoubleRow

DIM = 768
H = 768
G = 4 * H            # 3072, gate order reordered to [i, f, o, g]
B, T = 4, 2000
WIDTH, STRIDE = 200, 100
NFR = 20             # frames per batch element
NSEQ = B * NFR       # 80
NCORES = 8
SEQ_PC = NSEQ // NCORES   # 10
ROWS = SEQ_PC * WIDTH     # 2000 rows per core, row = t*10 + s (t-major)
MT = (ROWS + 127) // 128  # 16 row m-tiles (last has 80 rows)
CH_STEPS = 10             # timesteps per xw chunk DMA
NCH = WIDTH // CH_STEPS   # 20 chunks

SIG = mybir.ActivationFunctionType.Sigmoid
TANH = mybir.ActivationFunctionType.Tanh
MUL = mybir.AluOpType.mult
ADD = mybir.AluOpType.add

_CACHE = {}


def _mrows(m):
    return min(128, ROWS - m * 128)


def _build_program():
    nc = bacc.Bacc("TRN2", target_bir_lowering=False, debug=False,
                   num_devices=NCORES)

    xT_d = nc.declare_dram_parameter("xT", [DIM, ROWS], FP8, isOutput=False)
    wx0_d = nc.declare_dram_parameter("wx0", [2, DIM, G], FP8, isOutput=False)
    wh0_d = nc.declare_dram_parameter("wh0", [2, H, G], BF16, isOutput=False)
    b0_d = nc.declare_dram_parameter("b0", [2, 128, 24], F32, isOutput=False)
    wx1_d = nc.declare_dram_parameter("wx1", [2, 2 * H, G], FP8,
                                      isOutput=False)
    wh1_d = nc.declare_dram_parameter("wh1", [2, H, G], BF16, isOutput=False)
    b1_d = nc.declare_dram_parameter("b1", [2, 128, 24], F32, isOutput=False)
    linw_d = nc.declare_dram_parameter("linw", [2 * H, DIM], BF16,
                                       isOutput=False)
    linb_d = nc.declare_dram_parameter("linb", [DIM], F32, isOutput=False)
    out_d = nc.declare_dram_parameter("out", [ROWS, DIM], F32, isOutput=True)

    # xw stored transposed: [dir, gate m-tile, partition(gate%128), row]
    xw0_d = nc.dram_tensor("xw0", [2, 24, 128, ROWS], BF16)
    xw1_d = nc.dram_tensor("xw1", [2, 24, 128, ROWS], BF16)

    with TileContext(nc) as tc:
        with tc.tile_pool(name="const", bufs=1) as constp:
            identb = constp.tile([128, 128], BF16)
            make_identity(nc, identb[:])
            ones = constp.tile([1, 128], F32)
            nc.vector.memset(ones[:], 1.0)

            # ---- batched input projection, transposed output ----
            # xwT[d, m, p, row] = sum_k rhs_fn(row)[k] * Wx[k, m*128+p] + b
            # fp8 DoubleRow: each matmul consumes two 128-row K-tiles via
            # [128, 2, *] APs on both operands.
            def proj(kt, rhs_fn, wx_dram, b_dram, xw_dram, mid=None,
                     interleave=None):
                k2t = kt // 2
                for d in range(2):
                    if d == 1 and mid is not None:
                        mid()
                    with tc.tile_pool(name="wxp", bufs=1) as wxp, \
                         tc.tile_pool(name="bbp", bufs=1) as bbp, \
                         tc.tile_pool(name="pp", bufs=3, space="PSUM") as pp, \
                         tc.tile_pool(name="xo", bufs=3) as xop:
                        wx_sb = wxp.tile([128, kt, G], FP8)
                        for k in range(kt):
                            nc.sync.dma_start(
                                wx_sb[:, k],
                                wx_dram[d, k * 128:(k + 1) * 128, :])
                            if interleave is not None:
                                interleave(d, k)
                        bT = bbp.tile([128, 24], F32)
                        nc.sync.dma_start(bT[:], b_dram[d])
                        for m in range(24):
                            for r in range(4):
                                rs = slice(r * 500, (r + 1) * 500)
                                ps = pp.tile([128, 500], F32, tag="pp")
                                for k2 in range(k2t):
                                    nc.tensor.matmul(
                                        ps[:],
                                        wx_sb[:, 2 * k2:2 * k2 + 2,
                                              m * 128:(m + 1) * 128],
                                        rhs_fn(d, k2, rs),
                                        start=(k2 == 0), stop=(k2 == k2t - 1),
                                        perf_mode=DR)
                                xo = xop.tile([128, 500], BF16, tag="xo")
                                nc.vector.tensor_scalar(
                                    xo[:], ps[:], bT[:, m:m + 1], None, ADD)
                                nc.sync.dma_start(xw_dram[d, m, :, rs], xo[:])

            # ---- recurrence, both dirs interleaved, transposed layout ----
            # Gates split into two PSUM halves: [i,f] (m-tiles 0-11) and
            # [o,g] (m-tiles 12-23), so the sigmoid chain of a step starts
            # after only half its matmuls and hides under the rest.
            def recur(whp, wh_sb, xw_dram, hT, h8=None):
                with tc.tile_pool(name="stp", bufs=1) as stp, \
                     tc.tile_pool(name="xcp", bufs=2) as xcp, \
                     tc.tile_pool(name="gpp", bufs=2, space="PSUM") as gpp, \
                     tc.tile_pool(name="sgp", bufs=2) as sgp:
                    c = [stp.tile([128, 60], F32, name=f"c{d}")
                         for d in range(2)]
                    for d in range(2):
                        nc.vector.memset(c[d][:], 0.0)

                    for ch in range(NCH):
                        xc = []
                        for d in range(2):
                            cs = ch * 100 if d == 0 else 1900 - ch * 100
                            xct = xcp.tile([128, 24, 100], BF16,
                                           tag=f"xc{d}", name=f"xc{d}")
                            nc.sync.dma_start(
                                xct[:],
                                xw_dram[d, :, :, cs:cs + 100].rearrange(
                                    "m p c -> p m c"))
                            xc.append(xct)
                        for tl in range(CH_STEPS):
                            t = ch * CH_STEPS + tl
                            first = (t == 0)
                            sg, tcs = [], []
                            for d in range(2):
                                tt = t if d == 0 else WIDTH - 1 - t
                                lc = tl * 10 if d == 0 else (9 - tl) * 10
                                pv = (t - 1) * 10 if d == 0 else (tt + 1) * 10
                                gif = gpp.tile([128, 120], F32,
                                               tag=f"gi{d}", name=f"gi{d}")
                                gog = gpp.tile([128, 120], F32,
                                               tag=f"go{d}", name=f"go{d}")
                                for m in range(24):
                                    gg = gif if m < 12 else gog
                                    ms = slice((m % 12) * 10,
                                               (m % 12) * 10 + 10)
                                    nc.tensor.matmul(
                                        gg[:, ms],
                                        identb[:],
                                        xc[d][:, m, lc:lc + 10],
                                        start=True, stop=first)
                                    if not first:
                                        for k in range(6):
                                            nc.tensor.matmul(
                                                gg[:, ms],
                                                wh_sb[:, d, k,
                                                      m * 128:(m + 1) * 128],
                                                hT[d][:, k, pv:pv + 10],
                                                start=False, stop=(k == 5))
                                sgif = sgp.tile([128, 120], F32,
                                                tag=f"si{d}", name=f"si{d}")
                                tgg = sgp.tile([128, 60], F32,
                                               tag=f"tg{d}", name=f"tg{d}")
                                sgo = sgp.tile([128, 60], F32,
                                               tag=f"so{d}", name=f"so{d}")
                                tmm = sgp.tile([128, 60], F32,
                                               tag=f"tm{d}", name=f"tm{d}")
                                sg.append((sgif, sgo))
                                tcs.append(sgp.tile([128, 60], F32,
                                                    tag=f"tc{d}",
                                                    name=f"tc{d}"))
                                nc.scalar.activation(sgif[:], gif[:], SIG)
                                nc.scalar.activation(tgg[:], gog[:, 60:120],
                                                     TANH)
                                nc.scalar.activation(sgo[:], gog[:, 0:60],
                                                     SIG)
                                nc.gpsimd.tensor_tensor(
                                    tmm[:], sgif[:, 0:60], tgg[:], MUL)
                                nc.vector.tensor_tensor(
                                    c[d][:], c[d][:], sgif[:, 60:120], MUL)
                                nc.vector.tensor_tensor(
                                    c[d][:], c[d][:], tmm[:], ADD)
                            for d in range(2):
                                tt = t if d == 0 else WIDTH - 1 - t
                                nc.scalar.activation(tcs[d][:], c[d][:], TANH)
                                nc.vector.tensor_tensor(
                                    hT[d][:, :, tt * 10:(tt + 1) * 10],
                                    sg[d][1][:].rearrange(
                                        "p (a b) -> p a b", b=10),
                                    tcs[d][:].rearrange(
                                        "p (a b) -> p a b", b=10),
                                    MUL)
                            if h8 is not None:
                                for d in range(2):
                                    tt = t if d == 0 else WIDTH - 1 - t
                                    nc.gpsimd.tensor_tensor(
                                        h8[d][:, :, tt * 10:(tt + 1) * 10],
                                        sg[d][1][:].rearrange(
                                            "p (a b) -> p a b", b=10),
                                        tcs[d][:].rearrange(
                                            "p (a b) -> p a b", b=10),
                                        MUL)

            # ---- final linear: y = h1cat @ linW + linb ----
            def linear(hT1, lwp, lw, lbsb):
                with tc.tile_pool(name="lpp", bufs=3, space="PSUM") as lpp, \
                     tc.tile_pool(name="lop", bufs=2) as lop:
                    lbb = lwp.tile([128, DIM], F32)
                    for n in range(2):
                        ns = slice(n * 384, (n + 1) * 384)
                        bps = lpp.tile([128, 384], F32, tag="lp")
                        nc.tensor.matmul(bps[:], ones[:], lbsb[:, ns],
                                         start=True, stop=True)
                        nc.vector.tensor_copy(lbb[:, ns], bps[:])
                    for m in range(MT):
                        mr = _mrows(m)
                        lo = lop.tile([128, DIM], F32, tag="lo")
                        for n in range(2):
                            ns = slice(n * 384, (n + 1) * 384)
                            ps = lpp.tile([mr, 384], F32, tag="lp")
                            for k in range(12):
                                ht = hT1[0] if k < 6 else hT1[1]
                                kk = k if k < 6 else k - 6
                                nc.tensor.matmul(
                                    ps[:],
                                    ht[:, kk, m * 128:m * 128 + mr],
                                    lw[:, k, ns],
                                    start=(k == 0), stop=(k == 11))
                            nc.vector.tensor_tensor(
                                lo[:mr, ns], ps[:], lbb[:mr, ns], ADD)
                        nc.sync.dma_start(out_d[m * 128:m * 128 + mr, :],
                                          lo[:mr])

            def load_wh(whp, wh_dram, name):
                wh_sb = whp.tile([128, 2, 6, G], BF16, name=name)
                for d in range(2):
                    for k in range(6):
                        nc.sync.dma_start(
                            wh_sb[:, d, k],
                            wh_dram[d, k * 128:(k + 1) * 128, :])
                return wh_sb

            # h0/h1 share 2 slots: h1 reuses h0's space after proj1.
            with tc.tile_pool(name="hbp", bufs=2) as hbp, \
                 tc.tile_pool(name="h8p", bufs=1) as h8p:
                hT0 = [hbp.tile([128, 6, ROWS], BF16, tag="hb",
                                name=f"h0{d}") for d in range(2)]
                hT1 = [hbp.tile([128, 6, ROWS], BF16, tag="hb",
                                name=f"h1{d}") for d in range(2)]
                h08 = [h8p.tile([128, 6, ROWS], FP8, name=f"h08{d}")
                       for d in range(2)]
                with tc.tile_pool(name="whp0", bufs=1) as whp0:
                    wh0_box = []
                    with tc.tile_pool(name="xtp", bufs=1) as xtp:
                        xT_sb = xtp.tile([128, 6, ROWS], FP8)
                        proj(6,
                             lambda d, k2, rs:
                             xT_sb[:, 2 * k2:2 * k2 + 2, rs],
                             wx0_d, b0_d, xw0_d,
                             mid=lambda: wh0_box.append(
                                 load_wh(whp0, wh0_d, "wh0")),
                             interleave=lambda d, k:
                             nc.sync.dma_start(
                                 xT_sb[:, k],
                                 xT_d[k * 128:(k + 1) * 128, :])
                             if d == 0 else None)
                    recur(whp0, wh0_box[0], xw0_d, hT0, h8=h08)
                with tc.tile_pool(name="whp1", bufs=1) as whp1:
                    wh1_box = []
                    proj(12,
                         lambda d, k2, rs:
                         h08[0][:, 2 * k2:2 * k2 + 2, rs] if k2 < 3
                         else h08[1][:, 2 * k2 - 6:2 * k2 - 4, rs],
                         wx1_d, b1_d, xw1_d,
                         mid=lambda: wh1_box.append(
                             load_wh(whp1, wh1_d, "wh1")))
                    with tc.tile_pool(name="lwp", bufs=1) as lwp:
                        lw = lwp.tile([128, 12, DIM], BF16)
                        for k in range(12):
                            nc.sync.dma_start(
                                lw[:, k], linw_d[k * 128:(k + 1) * 128, :])
                        lbsb = lwp.tile([1, DIM], F32)
                        nc.sync.dma_start(lbsb[:], linb_d[None, :])
                        recur(whp1, wh1_box[0], xw1_d, hT1)
                        linear(hT1, lwp, lw, lbsb)

    nc.compile()
    return nc


def _reorder_gates(w):
    """[i f g o] -> [i f o g] along last axis (size 4H)."""
    i, f, g, o = np.split(w, 4, axis=-1)
    return np.concatenate([i, f, o, g], axis=-1)


def kernel(x, Wx0f, Wh0f, b0f, Wx0b, Wh0b, b0b,
           Wx1f, Wh1f, b1f, Wx1b, Wh1b, b1b, lin_W, lin_b):
    x = np.asarray(x, dtype=np.float32)
    # frame: (B, C, T) -> (NSEQ, DIM, WIDTH)
    tgt = (NFR - 1) * STRIDE + WIDTH
    xp = np.zeros((B, DIM, tgt), dtype=np.float32)
    xp[:, :, :T] = x
    frames = np.stack([xp[:, :, i:i + WIDTH]
                       for i in range(0, tgt - WIDTH + 1, STRIDE)], axis=1)
    xf = frames.reshape(NSEQ, DIM, WIDTH)

    def prepw(wf, wb, dt=BF16_NP):
        return np.ascontiguousarray(np.stack(
            [_reorder_gates(np.asarray(wf, np.float32)),
             _reorder_gates(np.asarray(wb, np.float32))])).astype(dt)

    def prepb(bf, bb_):
        # transposed bias: [dir, partition(gate%128), gate m-tile]
        return np.ascontiguousarray(np.stack(
            [_reorder_gates(np.asarray(bf, np.float32)).reshape(24, 128).T,
             _reorder_gates(np.asarray(bb_, np.float32)).reshape(24, 128).T]))

    wx0 = prepw(Wx0f, Wx0b, FP8_NP)
    wh0 = prepw(Wh0f, Wh0b)
    b0 = prepb(b0f, b0b)
    wx1 = prepw(Wx1f, Wx1b, FP8_NP)
    wh1 = prepw(Wh1f, Wh1b)
    b1 = prepb(b1f, b1b)
    linw = np.ascontiguousarray(np.asarray(lin_W, np.float32)).astype(BF16_NP)
    linb = np.ascontiguousarray(np.asarray(lin_b, np.float32))

    if "nc" not in _CACHE:
        _CACHE["nc"] = _build_program()
    nc = _CACHE["nc"]

    in_maps = []
    for cc in range(NCORES):
        shard = xf[cc * SEQ_PC:(cc + 1) * SEQ_PC]       # (10, 768, 200)
        xT = shard.transpose(1, 2, 0).reshape(DIM, ROWS)  # col = t*10 + s
        in_maps.append({"xT": np.ascontiguousarray(xT).astype(FP8_NP),
                        "wx0": wx0, "wh0": wh0, "b0": b0,
                        "wx1": wx1, "wh1": wh1, "b1": b1,
                        "linw": linw, "linb": linb})
    _CACHE["in_maps"] = in_maps

    res = run_bass_kernel_spmd(nc, in_maps, list(range(NCORES)))
    outs = [np.asarray(res.results[cc]["out"], np.float32)
            .reshape(WIDTH, SEQ_PC, DIM).transpose(1, 0, 2)
            for cc in range(NCORES)]                     # (10, 200, 768)
    y = np.concatenate(outs, axis=0)                     # (80, 200, 768)
    y = y.transpose(0, 2, 1).reshape(B, NFR, DIM, WIDTH)

    limit = STRIDE // 2
    parts = [y[:, 0, :, :-limit]]
    for k in range(1, NFR - 1):
        parts.append(y[:, k, :, limit:-limit])
    parts.append(y[:, NFR - 1, :, limit:])
    yc = np.concatenate(parts, axis=-1)[:, :, :T]        # (4, 768, 2000)
    return (yc + x).astype(np.float32)
